# revision 10
# baseline (speedup 1.0000x reference)
"""Trainium2 Bass kernel for nn_GAT_68642167324863 (5-layer dense GAT, N=2048).

Self-contained: hardcodes shapes/sharding. Shards the NxN edge grid by
destination row across 8 NeuronCores; node features are all-gathered between
layers via a DRAM-bounce AllGather collective.
"""
import sys
import numpy as np

sys.path.insert(0, "/opt/trn_rl_repo")

import concourse.bass as bass  # noqa: E402
import concourse.tile as tile  # noqa: E402
from concourse import bacc, mybir  # noqa: E402
from concourse import bass_utils  # noqa: E402

N = 2048          # nodes
NC = 8            # cores
SH = N // NC      # 256 rows per core
IT = SH // 128    # 2 i-tiles per core
H = 3             # heads
E = 3             # raw edge features
F32 = mybir.dt.float32
BF16 = mybir.dt.bfloat16
I32 = mybir.dt.int32
AF = mybir.ActivationFunctionType
OP = mybir.AluOpType

_CACHE = {}


def _host_consts(params):
    """Derive per-layer constant matrices from the GAT params (host math)."""
    P = {k: ({kk: np.asarray(vv, np.float32) for kk, vv in v.items()}
             if isinstance(v, dict) else np.float32(np.asarray(v)))
         for k, v in params.items()}
    L = []
    for li in range(5):
        p = P[f"l{li}"]
        We, be, a = p["We"], p["be"], p["a"]
        c = We.shape[0] // H                      # 3 for l0..3, 1 for l4
        if li == 0:
            a1 = a2 = None
            a3 = a                                 # [H, c]
        else:
            a1, a2, a3 = a[:, :c], a[:, c:2 * c], a[:, 2 * c:]
        W3 = np.zeros((E, H), np.float32)
        b3 = np.zeros((H,), np.float32)
        for h in range(H):
            for cc in range(c):
                W3[:, h] += a3[h, cc] * We[h * c + cc, :]
                b3[h] += a3[h, cc] * be[h * c + cc]
        WeBD = np.zeros((H * E, H * c), np.float32)
        for h in range(H):
            for cc in range(c):
                WeBD[h * E:(h + 1) * E, h * c + cc] = We[h * c + cc, :]
        ent = dict(c=c, W3=W3, b3=b3, WeBD=WeBD, be=be)
        if li > 0:
            Wn, bn = p["Wn"], p["bn"]             # [H*c, 9]
            d_in = Wn.shape[1]
            A1 = np.zeros((H, d_in), np.float32)
            A2 = np.zeros((H, d_in), np.float32)
            c1 = np.zeros((H,), np.float32)
            c2 = np.zeros((H,), np.float32)
            for h in range(H):
                for cc in range(c):
                    A1[h] += a1[h, cc] * Wn[h * c + cc]
                    A2[h] += a2[h, cc] * Wn[h * c + cc]
                    c1[h] += a1[h, cc] * bn[h * c + cc]
                    c2[h] += a2[h, cc] * bn[h * c + cc]
            c1 = c1 + b3                           # fold b3 into the a1 shift
            HCn = H * c
            PROJ = np.zeros((d_in, HCn + 6), np.float32)
            PROJ[:, :HCn] = Wn.T
            PROJ[:, HCn:HCn + 3] = A2.T
            PROJ[:, HCn + 3:] = A1.T
            BIAS = np.concatenate([bn, c2, c1]).astype(np.float32)
            ent.update(PROJ=PROJ, BIAS=BIAS, HCn=HCn)
        L.append(ent)
    pw = [float(np.asarray(P[f"p{i}"])) for i in range(4)]
    return L, pw


def _pack_wconst(L):
    blob, off = [], {}

    def put(name, arr):
        a = np.asarray(arr, np.float32).ravel()
        off[name] = (sum(x.size for x in blob), a.size)
        blob.append(a)

    for li in range(5):
        put(f"WeBD{li}", L[li]["WeBD"])
        c = L[li]["c"]
        for h in range(H):
            put(f"be{li}_{h}", L[li]["be"][h * c:(h + 1) * c])
        if li > 0:
            put(f"PROJ{li}", L[li]["PROJ"])
            put(f"BIAS{li}", L[li]["BIAS"])
    put("third", np.full((H, 1), 1.0 / H, np.float32))
    return np.concatenate(blob), off


def _build(L, pw, woff, wconst_len):
    nc = bacc.Bacc("TRN2", target_bir_lowering=False, debug=False,
                   num_devices=NC)
    ef_d = nc.dram_tensor("ef", [SH, N * E], F32, kind="ExternalInput").ap()
    adj_d = nc.dram_tensor("adj", [SH, N], I32, kind="ExternalInput").ap()
    wc_d = nc.dram_tensor("wc", [1, wconst_len], F32, kind="ExternalInput").ap()
    id_d = nc.dram_tensor("ident", [128, 128], F32, kind="ExternalInput").ap()
    out_d = nc.dram_tensor("out", [1, SH], F32, kind="ExternalOutput").ap()

    def wdram(name, shape):
        o, sz = woff[name]
        assert sz == int(np.prod(shape)), (name, shape, sz)
        ap = wc_d[0, o:o + sz]
        if len(shape) == 2:
            ap = ap.rearrange("(a b) -> a b", b=shape[1])
        else:
            ap = ap.rearrange("(a b) -> a b", b=1)
        return ap

    with tile.TileContext(nc) as tc:
        _emit(tc, nc, ef_d, adj_d, wdram, id_d, out_d, L, pw)
    nc.compile()
    return nc


def _emit(tc, nc, ef_d, adj_d, wdram, id_d, out_d, L, pw):
    from contextlib import ExitStack
    ctx = ExitStack()
    with ctx:
        persist = ctx.enter_context(tc.tile_pool(name="persist", bufs=1))
        consts = ctx.enter_context(tc.tile_pool(name="consts", bufs=1))
        a2pool = ctx.enter_context(tc.tile_pool(name="a2pool", bufs=1))
        lpool = ctx.enter_context(tc.tile_pool(name="lpool", bufs=3))
        upool = ctx.enter_context(tc.tile_pool(name="upool", bufs=2))
        ppool = ctx.enter_context(tc.tile_pool(name="ppool", bufs=3))
        mpool = ctx.enter_context(tc.tile_pool(name="mpool", bufs=3))
        tpool = ctx.enter_context(tc.tile_pool(name="tpool", bufs=2))
        ptpool = ctx.enter_context(tc.tile_pool(name="ptpool", bufs=3))
        smpool = ctx.enter_context(tc.tile_pool(name="smpool", bufs=1))
        accpool = ctx.enter_context(tc.tile_pool(name="accpool", bufs=2))
        psum = ctx.enter_context(tc.tile_pool(name="psum", bufs=1, space="PSUM"))
        psum_t = ctx.enter_context(tc.tile_pool(name="psum_t", bufs=1,
                                                space="PSUM"))
        dram = ctx.enter_context(tc.tile_pool(name="dram", bufs=1, space="DRAM"))

        ident = consts.tile([128, 128], F32)
        nc.sync.dma_start(ident[:], id_d[:])
        third_sb = consts.tile([H, 1], F32, tag="third")
        nc.sync.dma_start(third_sb[:], wdram("third", (H, 1)))

        # per-layer small consts into SBUF
        webd_sb, be_sb, proj_sb, bias_sb = {}, {}, {}, {}
        for li in range(5):
            hc = H * L[li]["c"]
            t = consts.tile([9, 16], F32, tag=f"webd{li}", name=f"webd{li}")
            nc.sync.dma_start(t[:, :hc], wdram(f"WeBD{li}", (9, hc)))
            webd_sb[li] = t
            cch = L[li]["c"]
            for h in range(H):
                t = consts.tile([3, 1], F32, tag=f"be{li}_{h}",
                                name=f"be{li}_{h}")
                nc.sync.dma_start(t[:cch, :], wdram(f"be{li}_{h}", (cch, 1)))
                be_sb[(li, h)] = t
            if li > 0:
                ncol = L[li]["HCn"] + 6
                t = consts.tile([9, 16], F32, tag=f"proj{li}", name=f"proj{li}")
                nc.sync.dma_start(t[:, :ncol], wdram(f"PROJ{li}", (9, ncol)))
                proj_sb[li] = t
                t = consts.tile([16, 1], F32, tag=f"bias{li}", name=f"bias{li}")
                nc.sync.dma_start(t[:ncol, :], wdram(f"BIAS{li}", (ncol, 1)))
                bias_sb[li] = t

        # ---- one-time prep: ef planes (bf16) + adjacency mask (bf16) ----
        efpl = persist.tile([128, IT, E, N], BF16)
        adjm = persist.tile([128, IT, N], BF16)
        with tc.tile_pool(name="stage", bufs=1) as stage:
            efv = ef_d.rearrange("(it p) q -> it p q", p=128)
            adv = adj_d.rearrange("(it p) q -> it p q", p=128)
            for it in range(IT):
                efst = stage.tile([128, N * E], F32, tag="efst")
                nc.sync.dma_start(efst[:], efv[it])
                adst = stage.tile([128, N], I32, tag="adst")
                nc.sync.dma_start(adst[:], adv[it])
                src = efst[:].rearrange("p (j e) -> p j e", e=E)
                for e in range(E):
                    nc.scalar.copy(efpl[:, it, e, :], src[:, :, e])
                nc.vector.tensor_copy(adjm[:, it, :], adst[:])

        # persistent cross-layer state
        x_new = persist.tile([9, SH], F32)           # [hc, i_local]
        alpha1 = persist.tile([128, IT, H], F32)     # local a1 shifts (+b3)
        ntr = persist.tile([128, NC, 2, 9], BF16)    # n by j-partition
        pay = dram.tile([1, 3072], BF16)
        gath = dram.tile([1, NC * 3072], BF16)

        for li in range(5):
            ly = L[li]
            c = ly["c"]
            W3, b3 = ly["W3"], ly["b3"]
            has_node = li > 0
            hc = H * c

            # ---- consume gathered node data (layers 1..4) ----
            if has_node:
                gv = gath[0, :].rearrange("(m q) -> m q", q=3072)
                for b in range(2):
                    nc.sync.dma_start(
                        ntr[:, :, b, :hc],
                        gv[:, b * 128 * hc:(b + 1) * 128 * hc].rearrange(
                            "m (p q) -> p m q", p=128))
                a2m = a2pool.tile([128, H, N], BF16)
                for h in range(H):
                    src = gv[:, 2304 + h * SH:2304 + (h + 1) * SH]
                    bsrc = bass.AP(tensor=src.tensor, offset=src.offset,
                                   ap=[[0, 128]] + list(src.ap))
                    nc.sync.dma_start(
                        a2m[:, h, :].rearrange("p (m i) -> p m i", m=NC),
                        bsrc)

            # accumulators: cols 0:9 = S (h-major,E), 9:12 = Z
            sacc = accpool.tile([128, IT, 12], F32)
            tps = ([psum_t.tile([4, SH], F32, tag=f"tps{h}",
                                 name=f"tps{li}_{h}")
                    for h in range(H)] if has_node else None)

            for it in range(IT):
                for h in range(H):
                    lg = lpool.tile([128, N], BF16)
                    if has_node:
                        nc.vector.scalar_tensor_tensor(
                            out=lg[:], in0=efpl[:, it, 0, :],
                            scalar=float(W3[0, h]), in1=a2m[:, h, :],
                            op0=OP.mult, op1=OP.add)
                    else:
                        nc.vector.tensor_scalar(
                            out=lg[:], in0=efpl[:, it, 0, :],
                            scalar1=float(W3[0, h]), scalar2=float(b3[h]),
                            op0=OP.mult, op1=OP.add)
                    lg2 = lpool.tile([128, N], BF16)
                    nc.vector.scalar_tensor_tensor(
                        out=lg2[:], in0=efpl[:, it, 1, :],
                        scalar=float(W3[1, h]), in1=lg[:],
                        op0=OP.mult, op1=OP.add)
                    lg3 = lpool.tile([128, N], BF16)
                    nc.vector.scalar_tensor_tensor(
                        out=lg3[:], in0=efpl[:, it, 2, :],
                        scalar=float(W3[2, h]), in1=lg2[:],
                        op0=OP.mult, op1=OP.add)
                    lr = upool.tile([128, N], BF16)
                    if has_node:
                        sh_ap = alpha1[:, it, h:h + 1]
                        u = upool.tile([128, N], BF16, tag="u2")
                        nc.vector.tensor_scalar(
                            out=u[:], in0=lg3[:], scalar1=sh_ap,
                            scalar2=0.2, op0=OP.add, op1=OP.mult)
                        nc.vector.scalar_tensor_tensor(
                            out=lr[:], in0=lg3[:], scalar=sh_ap,
                            in1=u[:], op0=OP.add, op1=OP.max)
                    else:
                        nc.vector.scalar_tensor_tensor(
                            out=lr[:], in0=lg3[:], scalar=0.2,
                            in1=lg3[:], op0=OP.mult, op1=OP.max)
                    pt = ppool.tile([128, N], BF16)
                    nc.scalar.activation(pt[:], lr[:], AF.Exp)
                    ph = mpool.tile([128, N], BF16)
                    nc.vector.scalar_tensor_tensor(
                        out=ph[:], in0=pt[:], scalar=1.0,
                        in1=adjm[:, it, :], op0=OP.mult, op1=OP.mult,
                        accum_out=sacc[:, it, 9 + h:10 + h])
                    for e in range(E):
                        tr = tpool.tile([128, N], BF16)
                        nc.vector.scalar_tensor_tensor(
                            out=tr[:], in0=ph[:], scalar=1.0,
                            in1=efpl[:, it, e, :], op0=OP.mult, op1=OP.mult,
                            accum_out=sacc[:, it, h * E + e:h * E + e + 1])
                    if has_node:
                        pht = ptpool.tile([128, NC * 2, 128], BF16)
                        nc.sync.dma_start_transpose(pht[:], ph[:])
                        for blk in range(NC * 2):
                            nc.tensor.matmul(
                                tps[h][:c, it * 128:(it + 1) * 128],
                                ntr[:, blk // 2, blk % 2,
                                    h * c:(h + 1) * c],
                                pht[:, blk, :],
                                start=(blk == 0), stop=(blk == NC * 2 - 1))

            # ---- assembly (per-head, partition-base-0 tiles) ----
            zinv = accpool.tile([128, IT, H], F32, tag="zinv")
            nc.vector.reciprocal(zinv[:, :, :], sacc[:, :, 9:12])
            zrep = accpool.tile([128, IT, hc], F32, tag="zrep")
            if c > 1:
                zv = zinv[:, :, :]
                src = bass.AP(tensor=zv.tensor, offset=zv.offset,
                              ap=list(zv.ap) + [[0, c]])
                nc.vector.tensor_copy(
                    zrep[:].rearrange("p a (h cc) -> p a h cc", cc=c), src)
            else:
                nc.vector.tensor_copy(zrep[:], zinv[:, :, :])
            st_sb = smpool.tile([9, SH], F32)
            for it in range(IT):
                st_ps = psum.tile([12, 128], F32, tag="stp")
                nc.tensor.transpose(st_ps[:9, :], sacc[:, it, 0:9], ident[:])
                nc.scalar.copy(st_sb[:, it * 128:(it + 1) * 128], st_ps[:9, :])
            zts = []
            for h in range(H):
                zt_sb = smpool.tile([4, SH], F32, tag=f"zts{h}",
                                    name=f"zts{li}_{h}")
                for it in range(IT):
                    zt_ps = psum.tile([4, 128], F32, tag="ztp")
                    nc.tensor.transpose(zt_ps[:c, :],
                                        zrep[:, it, h * c:(h + 1) * c],
                                        ident[:])
                    nc.scalar.copy(zt_sb[:c, it * 128:(it + 1) * 128],
                                   zt_ps[:c, :])
                zts.append(zt_sb)
            xnh = []
            for h in range(H):
                wes_ps = psum.tile([4, SH], F32, tag="asm")
                nc.tensor.matmul(wes_ps[:c, :],
                                 webd_sb[li][:, h * c:(h + 1) * c],
                                 st_sb[:], start=True, stop=True)
                xp = smpool.tile([4, SH], F32, tag="xp")
                nc.scalar.copy(xp[:c, :], wes_ps[:c, :])
                if has_node:
                    xs = smpool.tile([4, SH], F32, tag="xs")
                    nc.vector.scalar_tensor_tensor(
                        out=xs[:c, :], in0=tps[h][:c, :], scalar=1.0,
                        in1=xp[:c, :], op0=OP.mult, op1=OP.add)
                else:
                    xs = xp
                xz = smpool.tile([4, SH], F32, tag="xz")
                nc.vector.tensor_tensor(xz[:c, :], xs[:c, :], zts[h][:c, :],
                                        op=OP.mult)
                xnh.append(xz)
            if li < 4:
                w = pw[li]
                for h in range(H):
                    beh = be_sb[(li, h)]
                    xz = xnh[h]
                    u2 = smpool.tile([4, SH], F32, tag="u3")
                    nc.vector.tensor_scalar(
                        out=u2[:c, :], in0=xz[:c, :], scalar1=beh[:c, :],
                        scalar2=w, op0=OP.add, op1=OP.mult)
                    xn = smpool.tile([4, SH], F32, tag="xn")
                    nc.vector.scalar_tensor_tensor(
                        out=xn[:c, :], in0=xz[:c, :], scalar=beh[:c, :],
                        in1=u2[:c, :], op0=OP.add, op1=OP.max)
                    nc.sync.dma_start(x_new[h * c:(h + 1) * c, :], xn[:c, :])

                nly = L[li + 1]
                nHC = nly["HCn"]
                ncol = nHC + 6
                r_ps = psum.tile([16, SH], F32, tag="asm")
                nc.tensor.matmul(r_ps[:ncol, :], proj_sb[li + 1][:, :ncol],
                                 x_new[:], start=True, stop=True)
                r_sb = smpool.tile([16, SH], F32, tag="rsb")
                nc.scalar.activation(r_sb[:ncol, :], r_ps[:ncol, :],
                                     AF.Identity,
                                     bias=bias_sb[li + 1][:ncol, :])
                a2b = smpool.tile([16, SH], BF16, tag="a2b")
                nc.vector.tensor_copy(a2b[:ncol, :], r_sb[:ncol, :])
                nc.sync.dma_start(
                    pay[0, 2304:2304 + H * SH].rearrange(
                        "(h i) -> h i", h=H), a2b[nHC:nHC + 3, :])
                for it in range(IT):
                    rt_ps = psum.tile([128, 16], F32, tag="rtp")
                    nc.tensor.transpose(
                        rt_ps[:, :ncol],
                        r_sb[:ncol, it * 128:(it + 1) * 128],
                        ident[:ncol, :ncol])
                    nc.scalar.copy(alpha1[:, it, :],
                                   rt_ps[:, nHC + 3:nHC + 6])
                    nrm = smpool.tile([128, 16], BF16, tag="nrm")
                    nc.vector.tensor_copy(nrm[:, :nHC], rt_ps[:, :nHC])
                    nc.sync.dma_start(
                        pay[0, it * 128 * nHC:(it + 1) * 128 * nHC].rearrange(
                            "(p q) -> p q", p=128), nrm[:, :nHC])
                nc.gpsimd.collective_compute(
                    "AllGather", OP.bypass,
                    replica_groups=[list(range(NC))],
                    ins=[pay[:].opt()], outs=[gath[:].opt()])
            else:
                xh3 = smpool.tile([H, SH], F32, tag="xh3")
                for h in range(H):
                    xb = smpool.tile([4, SH], F32, tag="xb")
                    nc.vector.tensor_scalar(
                        out=xb[:c, :], in0=xnh[h][:c, :],
                        scalar1=be_sb[(li, h)][:c, :],
                        scalar2=None, op0=OP.add)
                    nc.sync.dma_start(xh3[h:h + 1, :], xb[:c, :])
                mn_ps = psum.tile([1, SH], F32, tag="mn")
                nc.tensor.matmul(mn_ps[:], third_sb[:], xh3[:],
                                 start=True, stop=True)
                x4 = smpool.tile([1, SH], F32, tag="x4")
                nc.scalar.copy(x4[:], mn_ps[:])
                zz = smpool.tile([1, SH], F32, tag="zz")
                nc.vector.tensor_tensor(zz[:], x4[:], x4[:], op=OP.subtract)
                ex = smpool.tile([1, SH], F32, tag="ex")
                nc.scalar.activation(ex[:], zz[:], AF.Exp)
                exr = smpool.tile([1, SH], F32, tag="exr")
                nc.vector.reciprocal(exr[:], ex[:])
                ones = smpool.tile([1, SH], F32, tag="ones")
                nc.vector.tensor_tensor(ones[:], ex[:], exr[:], op=OP.mult)
                nc.sync.dma_start(out_d[:], ones[:])


def _get_program(params):
    key = "prog"
    if key not in _CACHE:
        L, pw = _host_consts(params)
        wc, woff = _pack_wconst(L)
        nc = _build(L, pw, woff, wc.size)
        _CACHE[key] = (nc, wc)
    return _CACHE[key]


def make_in_maps(edge_feats, adj_matrix, params):
    ef = np.asarray(edge_feats, np.float32).reshape(N, N * E)
    adj = np.ascontiguousarray(np.asarray(adj_matrix, np.int32).reshape(N, N))
    nc, wc = _get_program(params)
    ident = np.eye(128, dtype=np.float32)
    in_maps = []
    for k in range(NC):
        in_maps.append({
            "ef": np.ascontiguousarray(ef[k * SH:(k + 1) * SH]),
            "adj": adj[k * SH:(k + 1) * SH],
            "wc": wc.reshape(1, -1),
            "ident": ident,
        })
    return nc, in_maps


def kernel(edge_feats, adj_matrix, params):
    nc, in_maps = make_in_maps(edge_feats, adj_matrix, params)
    res = bass_utils.run_bass_kernel_spmd(nc, in_maps, core_ids=list(range(NC)))
    out = np.concatenate([res.results[k]["out"][0] for k in range(NC)])
    return out.reshape(1, N, 1).astype(np.float32)


# revision 13
# speedup vs baseline: 5.4696x; 5.4696x over previous
"""Trainium2 Bass kernel for nn_GAT_68642167324863 (5-layer dense GAT, N=2048).

Self-contained: hardcodes shapes/sharding. Shards the NxN edge grid by
destination row across 8 NeuronCores; node features are all-gathered between
layers via a DRAM-bounce AllGather collective.
"""
import sys
import numpy as np

sys.path.insert(0, "/opt/trn_rl_repo")

import concourse.bass as bass  # noqa: E402
import concourse.tile as tile  # noqa: E402
from concourse import bacc, mybir  # noqa: E402
from concourse import bass_utils  # noqa: E402

N = 2048          # nodes
NC = 8            # cores
SH = N // NC      # 256 rows per core
IT = SH // 128    # 2 i-tiles per core
H = 3             # heads
E = 3             # raw edge features
F32 = mybir.dt.float32
BF16 = mybir.dt.bfloat16
I32 = mybir.dt.int32
AF = mybir.ActivationFunctionType
OP = mybir.AluOpType

_CACHE = {}


def _host_consts(params):
    """Derive per-layer constant matrices from the GAT params (host math)."""
    P = {k: ({kk: np.asarray(vv, np.float32) for kk, vv in v.items()}
             if isinstance(v, dict) else np.float32(np.asarray(v)))
         for k, v in params.items()}
    L = []
    for li in range(5):
        p = P[f"l{li}"]
        We, be, a = p["We"], p["be"], p["a"]
        c = We.shape[0] // H                      # 3 for l0..3, 1 for l4
        if li == 0:
            a1 = a2 = None
            a3 = a                                 # [H, c]
        else:
            a1, a2, a3 = a[:, :c], a[:, c:2 * c], a[:, 2 * c:]
        W3 = np.zeros((E, H), np.float32)
        b3 = np.zeros((H,), np.float32)
        for h in range(H):
            for cc in range(c):
                W3[:, h] += a3[h, cc] * We[h * c + cc, :]
                b3[h] += a3[h, cc] * be[h * c + cc]
        WeBD = np.zeros((H * E, H * c), np.float32)
        for h in range(H):
            for cc in range(c):
                WeBD[h * E:(h + 1) * E, h * c + cc] = We[h * c + cc, :]
        ent = dict(c=c, W3=W3, b3=b3, WeBD=WeBD, be=be)
        if li > 0:
            Wn, bn = p["Wn"], p["bn"]             # [H*c, 9]
            d_in = Wn.shape[1]
            A1 = np.zeros((H, d_in), np.float32)
            A2 = np.zeros((H, d_in), np.float32)
            c1 = np.zeros((H,), np.float32)
            c2 = np.zeros((H,), np.float32)
            for h in range(H):
                for cc in range(c):
                    A1[h] += a1[h, cc] * Wn[h * c + cc]
                    A2[h] += a2[h, cc] * Wn[h * c + cc]
                    c1[h] += a1[h, cc] * bn[h * c + cc]
                    c2[h] += a2[h, cc] * bn[h * c + cc]
            c1 = c1 + b3                           # fold b3 into the a1 shift
            HCn = H * c
            PROJ = np.zeros((d_in, HCn + 6), np.float32)
            PROJ[:, :HCn] = Wn.T
            PROJ[:, HCn:HCn + 3] = A2.T
            PROJ[:, HCn + 3:] = A1.T
            BIAS = np.concatenate([bn, c2, c1]).astype(np.float32)
            ent.update(PROJ=PROJ, BIAS=BIAS, HCn=HCn)
        L.append(ent)
    pw = [float(np.asarray(P[f"p{i}"])) for i in range(4)]
    return L, pw


def _pack_wconst(L):
    blob, off = [], {}

    def put(name, arr):
        a = np.asarray(arr, np.float32).ravel()
        off[name] = (sum(x.size for x in blob), a.size)
        blob.append(a)

    for li in range(5):
        put(f"WeBD{li}", L[li]["WeBD"])
        c = L[li]["c"]
        for h in range(H):
            put(f"be{li}_{h}", L[li]["be"][h * c:(h + 1) * c])
        if li > 0:
            put(f"PROJ{li}", L[li]["PROJ"])
            put(f"BIAS{li}", L[li]["BIAS"])
    put("third", np.full((H, 1), 1.0 / H, np.float32))
    return np.concatenate(blob), off


def _build(L, pw, woff, wconst_len):
    nc = bacc.Bacc("TRN2", target_bir_lowering=False, debug=False,
                   num_devices=NC)
    ef_d = nc.dram_tensor("ef", [SH, N * E], F32, kind="ExternalInput").ap()
    adj_d = nc.dram_tensor("adj", [SH, N], I32, kind="ExternalInput").ap()
    wc_d = nc.dram_tensor("wc", [1, wconst_len], F32, kind="ExternalInput").ap()
    id_d = nc.dram_tensor("ident", [128, 128], F32, kind="ExternalInput").ap()
    out_d = nc.dram_tensor("out", [1, SH], F32, kind="ExternalOutput").ap()

    def wdram(name, shape):
        o, sz = woff[name]
        assert sz == int(np.prod(shape)), (name, shape, sz)
        ap = wc_d[0, o:o + sz]
        if len(shape) == 2:
            ap = ap.rearrange("(a b) -> a b", b=shape[1])
        else:
            ap = ap.rearrange("(a b) -> a b", b=1)
        return ap

    with tile.TileContext(nc) as tc:
        _emit(tc, nc, ef_d, adj_d, wdram, id_d, out_d, L, pw)
    nc.compile()
    return nc


def _emit(tc, nc, ef_d, adj_d, wdram, id_d, out_d, L, pw):
    from contextlib import ExitStack
    ctx = ExitStack()
    with ctx:
        persist = ctx.enter_context(tc.tile_pool(name="persist", bufs=1))
        consts = ctx.enter_context(tc.tile_pool(name="consts", bufs=1))
        a2pool = ctx.enter_context(tc.tile_pool(name="a2pool", bufs=1))
        lpool = ctx.enter_context(tc.tile_pool(name="lpool", bufs=3))
        upool = ctx.enter_context(tc.tile_pool(name="upool", bufs=1))
        ppool = ctx.enter_context(tc.tile_pool(name="ppool", bufs=2))
        mpool = ctx.enter_context(tc.tile_pool(name="mpool", bufs=2))
        tpool = ctx.enter_context(tc.tile_pool(name="tpool", bufs=2))
        ptpool = ctx.enter_context(tc.tile_pool(name="ptpool", bufs=3))
        smpool = ctx.enter_context(tc.tile_pool(name="smpool", bufs=1))
        accpool = ctx.enter_context(tc.tile_pool(name="accpool", bufs=2))
        psum = ctx.enter_context(tc.tile_pool(name="psum", bufs=1, space="PSUM"))
        psum_t = ctx.enter_context(tc.tile_pool(name="psum_t", bufs=1,
                                                space="PSUM"))
        dram = ctx.enter_context(tc.tile_pool(name="dram", bufs=1, space="DRAM"))

        ident = consts.tile([128, 128], F32)
        nc.sync.dma_start(ident[:], id_d[:])
        third_sb = consts.tile([H, 1], F32, tag="third")
        nc.sync.dma_start(third_sb[:], wdram("third", (H, 1)))

        # per-layer small consts into SBUF
        webd_sb, be_sb, proj_sb, bias_sb = {}, {}, {}, {}
        for li in range(5):
            hc = H * L[li]["c"]
            t = consts.tile([9, 16], F32, tag=f"webd{li}", name=f"webd{li}")
            nc.sync.dma_start(t[:, :hc], wdram(f"WeBD{li}", (9, hc)))
            webd_sb[li] = t
            cch = L[li]["c"]
            for h in range(H):
                t = consts.tile([3, 1], F32, tag=f"be{li}_{h}",
                                name=f"be{li}_{h}")
                nc.sync.dma_start(t[:cch, :], wdram(f"be{li}_{h}", (cch, 1)))
                be_sb[(li, h)] = t
            if li > 0:
                ncol = L[li]["HCn"] + 6
                t = consts.tile([9, 16], F32, tag=f"proj{li}", name=f"proj{li}")
                nc.sync.dma_start(t[:, :ncol], wdram(f"PROJ{li}", (9, ncol)))
                proj_sb[li] = t
                t = consts.tile([16, 1], F32, tag=f"bias{li}", name=f"bias{li}")
                nc.sync.dma_start(t[:ncol, :], wdram(f"BIAS{li}", (ncol, 1)))
                bias_sb[li] = t

        # ---- one-time prep: ef planes (bf16) + adjacency mask (bf16) ----
        efpl = persist.tile([128, IT, E, N], BF16)
        adjm = persist.tile([128, IT, N], BF16)
        with tc.tile_pool(name="stage", bufs=1) as stage:
            efv = ef_d.rearrange("(it p) (hf q) -> it p hf q", p=128, hf=2)
            adv = adj_d.rearrange("(it p) q -> it p q", p=128)
            for it in range(IT):
                for hf in range(2):
                    efst = stage.tile([128, N * E // 2], F32, tag="efst")
                    nc.sync.dma_start(efst[:], efv[it, :, hf, :])
                    src = efst[:].rearrange("p (j e) -> p j e", e=E)
                    for e in range(E):
                        nc.scalar.copy(
                            efpl[:, it, e, hf * (N // 2):(hf + 1) * (N // 2)],
                            src[:, :, e])
                adst = stage.tile([128, N], I32, tag="adst")
                nc.sync.dma_start(adst[:], adv[it])
                nc.vector.tensor_copy(adjm[:, it, :], adst[:])

        # persistent cross-layer state
        x_new = persist.tile([9, SH], F32)           # [hc, i_local]
        alpha1 = persist.tile([128, IT, H], F32)     # local a1 shifts (+b3)
        alpha1n = persist.tile([128, IT, H], F32)    # negated shifts
        ntr = persist.tile([128, NC, 2, 9], BF16)    # n by j-partition
        pay = dram.tile([1, 3072], BF16)
        gath = dram.tile([1, NC * 3072], BF16)

        for li in range(5):
            ly = L[li]
            c = ly["c"]
            W3, b3 = ly["W3"], ly["b3"]
            has_node = li > 0
            hc = H * c

            # ---- consume gathered node data (layers 1..4) ----
            if has_node:
                gv = gath[0, :].rearrange("(m q) -> m q", q=3072)
                for b in range(2):
                    nc.sync.dma_start(
                        ntr[:, :, b, :hc],
                        gv[:, b * 128 * hc:(b + 1) * 128 * hc].rearrange(
                            "m (p q) -> p m q", p=128))
                a2m = a2pool.tile([128, H, N], BF16)
                for h in range(H):
                    src = gv[:, 2304 + h * SH:2304 + (h + 1) * SH]
                    bsrc = bass.AP(tensor=src.tensor, offset=src.offset,
                                   ap=[[0, 128]] + list(src.ap))
                    nc.sync.dma_start(
                        a2m[:, h, :].rearrange("p (m i) -> p m i", m=NC),
                        bsrc)

            # accumulators: cols 0:9 = S (h-major,E), 9:12 = Z
            sacc = accpool.tile([128, IT, 12], F32)
            tps = ([psum_t.tile([4, SH], F32, tag=f"tps{h}",
                                 name=f"tps{li}_{h}")
                    for h in range(H)] if has_node else None)

            for it in range(IT):
                r1all = upool.tile([128, H, N], BF16, name="r1all")
                r2all = upool.tile([128, H, N], BF16, tag="u2", name="r2all")
                for h in range(H):
                    lg = lpool.tile([128, N], BF16)
                    if has_node:
                        nc.vector.scalar_tensor_tensor(
                            out=lg[:], in0=efpl[:, it, 0, :],
                            scalar=float(W3[0, h]), in1=a2m[:, h, :],
                            op0=OP.mult, op1=OP.add)
                    else:
                        nc.vector.tensor_scalar(
                            out=lg[:], in0=efpl[:, it, 0, :],
                            scalar1=float(W3[0, h]), scalar2=float(b3[h]),
                            op0=OP.mult, op1=OP.add)
                    lg2 = lpool.tile([128, N], BF16)
                    nc.vector.scalar_tensor_tensor(
                        out=lg2[:], in0=efpl[:, it, 1, :],
                        scalar=float(W3[1, h]), in1=lg[:],
                        op0=OP.mult, op1=OP.add)
                    if has_node:
                        lg3 = lpool.tile([128, N], BF16)
                        nc.vector.scalar_tensor_tensor(
                            out=lg3[:], in0=efpl[:, it, 2, :],
                            scalar=float(W3[2, h]), in1=lg2[:],
                            op0=OP.mult, op1=OP.add)
                        nc.scalar.activation(
                            r1all[:, h, :], lg3[:], AF.Relu,
                            bias=alpha1[:, it, h:h + 1], scale=1.0)
                        nc.scalar.activation(
                            r2all[:, h, :], lg3[:], AF.Relu,
                            bias=alpha1n[:, it, h:h + 1], scale=-1.0)
                    else:
                        nc.vector.scalar_tensor_tensor(
                            out=r1all[:, h, :], in0=efpl[:, it, 2, :],
                            scalar=float(W3[2, h]), in1=lg2[:],
                            op0=OP.mult, op1=OP.add)
                lrall = upool.tile([128, H, N], BF16, tag="u3", name="lrall")
                if has_node:
                    # lrelu(y) = relu(y) - 0.2*relu(-y)
                    nc.vector.scalar_tensor_tensor(
                        out=lrall[:].rearrange("p h n -> p (h n)"),
                        in0=r2all[:].rearrange("p h n -> p (h n)"),
                        scalar=-0.2,
                        in1=r1all[:].rearrange("p h n -> p (h n)"),
                        op0=OP.mult, op1=OP.add)
                else:
                    v = r1all[:].rearrange("p h n -> p (h n)")
                    nc.vector.scalar_tensor_tensor(
                        out=lrall[:].rearrange("p h n -> p (h n)"),
                        in0=v, scalar=0.2, in1=v, op0=OP.mult, op1=OP.max)
                for h in range(H):
                    pt = ppool.tile([128, N], BF16)
                    nc.scalar.activation(pt[:], lrall[:, h, :], AF.Exp)
                    ph = mpool.tile([128, N], BF16)
                    nc.vector.scalar_tensor_tensor(
                        out=ph[:], in0=pt[:], scalar=1.0,
                        in1=adjm[:, it, :], op0=OP.mult, op1=OP.mult,
                        accum_out=sacc[:, it, 9 + h:10 + h])
                    for e in range(E):
                        tr = tpool.tile([128, N], BF16)
                        nc.vector.scalar_tensor_tensor(
                            out=tr[:], in0=ph[:], scalar=1.0,
                            in1=efpl[:, it, e, :], op0=OP.mult, op1=OP.mult,
                            accum_out=sacc[:, it, h * E + e:h * E + e + 1])
                    if has_node:
                        pht = ptpool.tile([128, NC * 2, 128], BF16)
                        nc.sync.dma_start_transpose(pht[:], ph[:])
                        for blk in range(NC * 2):
                            nc.tensor.matmul(
                                tps[h][:c, it * 128:(it + 1) * 128],
                                ntr[:, blk // 2, blk % 2,
                                    h * c:(h + 1) * c],
                                pht[:, blk, :],
                                start=(blk == 0), stop=(blk == NC * 2 - 1))

            # ---- assembly (per-head, partition-base-0 tiles) ----
            zinv = accpool.tile([128, IT, H], F32, tag="zinv")
            nc.vector.reciprocal(zinv[:, :, :], sacc[:, :, 9:12])
            zrep = accpool.tile([128, IT, hc], F32, tag="zrep")
            if c > 1:
                zv = zinv[:, :, :]
                src = bass.AP(tensor=zv.tensor, offset=zv.offset,
                              ap=list(zv.ap) + [[0, c]])
                nc.vector.tensor_copy(
                    zrep[:].rearrange("p a (h cc) -> p a h cc", cc=c), src)
            else:
                nc.vector.tensor_copy(zrep[:], zinv[:, :, :])
            st_sb = smpool.tile([9, SH], F32)
            for it in range(IT):
                st_ps = psum.tile([12, 128], F32, tag="stp")
                nc.tensor.transpose(st_ps[:9, :], sacc[:, it, 0:9], ident[:])
                nc.scalar.copy(st_sb[:, it * 128:(it + 1) * 128], st_ps[:9, :])
            zts = []
            for h in range(H):
                zt_sb = smpool.tile([4, SH], F32, tag=f"zts{h}",
                                    name=f"zts{li}_{h}")
                for it in range(IT):
                    zt_ps = psum.tile([4, 128], F32, tag="ztp")
                    nc.tensor.transpose(zt_ps[:c, :],
                                        zrep[:, it, h * c:(h + 1) * c],
                                        ident[:])
                    nc.scalar.copy(zt_sb[:c, it * 128:(it + 1) * 128],
                                   zt_ps[:c, :])
                zts.append(zt_sb)
            xnh = []
            for h in range(H):
                wes_ps = psum.tile([4, SH], F32, tag="asm")
                nc.tensor.matmul(wes_ps[:c, :],
                                 webd_sb[li][:, h * c:(h + 1) * c],
                                 st_sb[:], start=True, stop=True)
                xp = smpool.tile([4, SH], F32, tag="xp")
                nc.scalar.copy(xp[:c, :], wes_ps[:c, :])
                if has_node:
                    xs = smpool.tile([4, SH], F32, tag="xs")
                    nc.vector.scalar_tensor_tensor(
                        out=xs[:c, :], in0=tps[h][:c, :], scalar=1.0,
                        in1=xp[:c, :], op0=OP.mult, op1=OP.add)
                else:
                    xs = xp
                xz = smpool.tile([4, SH], F32, tag="xz")
                nc.vector.tensor_tensor(xz[:c, :], xs[:c, :], zts[h][:c, :],
                                        op=OP.mult)
                xnh.append(xz)
            if li < 4:
                w = pw[li]
                for h in range(H):
                    beh = be_sb[(li, h)]
                    xz = xnh[h]
                    u2 = smpool.tile([4, SH], F32, tag="u3")
                    nc.vector.tensor_scalar(
                        out=u2[:c, :], in0=xz[:c, :], scalar1=beh[:c, :],
                        scalar2=w, op0=OP.add, op1=OP.mult)
                    xn = smpool.tile([4, SH], F32, tag="xn")
                    nc.vector.scalar_tensor_tensor(
                        out=xn[:c, :], in0=xz[:c, :], scalar=beh[:c, :],
                        in1=u2[:c, :], op0=OP.add, op1=OP.max)
                    nc.sync.dma_start(x_new[h * c:(h + 1) * c, :], xn[:c, :])

                nly = L[li + 1]
                nHC = nly["HCn"]
                ncol = nHC + 6
                r_ps = psum.tile([16, SH], F32, tag="asm")
                nc.tensor.matmul(r_ps[:ncol, :], proj_sb[li + 1][:, :ncol],
                                 x_new[:], start=True, stop=True)
                r_sb = smpool.tile([16, SH], F32, tag="rsb")
                nc.scalar.activation(r_sb[:ncol, :], r_ps[:ncol, :],
                                     AF.Identity,
                                     bias=bias_sb[li + 1][:ncol, :])
                a2b = smpool.tile([16, SH], BF16, tag="a2b")
                nc.vector.tensor_copy(a2b[:ncol, :], r_sb[:ncol, :])
                nc.sync.dma_start(
                    pay[0, 2304:2304 + H * SH].rearrange(
                        "(h i) -> h i", h=H), a2b[nHC:nHC + 3, :])
                for it in range(IT):
                    rt_ps = psum.tile([128, 16], F32, tag="rtp")
                    nc.tensor.transpose(
                        rt_ps[:, :ncol],
                        r_sb[:ncol, it * 128:(it + 1) * 128],
                        ident[:ncol, :ncol])
                    nc.scalar.copy(alpha1[:, it, :],
                                   rt_ps[:, nHC + 3:nHC + 6])
                    nc.vector.tensor_scalar(
                        out=alpha1n[:, it, :], in0=alpha1[:, it, :],
                        scalar1=-1.0, scalar2=None, op0=OP.mult)
                    nrm = smpool.tile([128, 16], BF16, tag="nrm")
                    nc.vector.tensor_copy(nrm[:, :nHC], rt_ps[:, :nHC])
                    nc.sync.dma_start(
                        pay[0, it * 128 * nHC:(it + 1) * 128 * nHC].rearrange(
                            "(p q) -> p q", p=128), nrm[:, :nHC])
                nc.gpsimd.collective_compute(
                    "AllGather", OP.bypass,
                    replica_groups=[list(range(NC))],
                    ins=[pay[:].opt()], outs=[gath[:].opt()])
            else:
                xh3 = smpool.tile([H, SH], F32, tag="xh3")
                for h in range(H):
                    xb = smpool.tile([4, SH], F32, tag="xb")
                    nc.vector.tensor_scalar(
                        out=xb[:c, :], in0=xnh[h][:c, :],
                        scalar1=be_sb[(li, h)][:c, :],
                        scalar2=None, op0=OP.add)
                    nc.sync.dma_start(xh3[h:h + 1, :], xb[:c, :])
                mn_ps = psum.tile([1, SH], F32, tag="mn")
                nc.tensor.matmul(mn_ps[:], third_sb[:], xh3[:],
                                 start=True, stop=True)
                x4 = smpool.tile([1, SH], F32, tag="x4")
                nc.scalar.copy(x4[:], mn_ps[:])
                zz = smpool.tile([1, SH], F32, tag="zz")
                nc.vector.tensor_tensor(zz[:], x4[:], x4[:], op=OP.subtract)
                ex = smpool.tile([1, SH], F32, tag="ex")
                nc.scalar.activation(ex[:], zz[:], AF.Exp)
                exr = smpool.tile([1, SH], F32, tag="exr")
                nc.vector.reciprocal(exr[:], ex[:])
                ones = smpool.tile([1, SH], F32, tag="ones")
                nc.vector.tensor_tensor(ones[:], ex[:], exr[:], op=OP.mult)
                nc.sync.dma_start(out_d[:], ones[:])


def _get_program(params):
    key = "prog"
    if key not in _CACHE:
        L, pw = _host_consts(params)
        wc, woff = _pack_wconst(L)
        nc = _build(L, pw, woff, wc.size)
        _CACHE[key] = (nc, wc)
    return _CACHE[key]


def make_in_maps(edge_feats, adj_matrix, params):
    ef = np.asarray(edge_feats, np.float32).reshape(N, N * E)
    adj = np.ascontiguousarray(np.asarray(adj_matrix, np.int32).reshape(N, N))
    nc, wc = _get_program(params)
    ident = np.eye(128, dtype=np.float32)
    in_maps = []
    for k in range(NC):
        in_maps.append({
            "ef": np.ascontiguousarray(ef[k * SH:(k + 1) * SH]),
            "adj": adj[k * SH:(k + 1) * SH],
            "wc": wc.reshape(1, -1),
            "ident": ident,
        })
    return nc, in_maps


def kernel(edge_feats, adj_matrix, params):
    nc, in_maps = make_in_maps(edge_feats, adj_matrix, params)
    res = bass_utils.run_bass_kernel_spmd(nc, in_maps, core_ids=list(range(NC)))
    out = np.concatenate([res.results[k]["out"][0] for k in range(NC)])
    return out.reshape(1, N, 1).astype(np.float32)


# revision 14
# speedup vs baseline: 10.5808x; 1.9345x over previous
"""Trainium2 Bass kernel for nn_GAT_68642167324863 (5-layer dense GAT, N=2048).

Self-contained: hardcodes shapes/sharding. Shards the NxN edge grid by
destination row across 8 NeuronCores; node features are all-gathered between
layers via a DRAM-bounce AllGather collective.
"""
import sys
import numpy as np

sys.path.insert(0, "/opt/trn_rl_repo")

import concourse.bass as bass  # noqa: E402
import concourse.tile as tile  # noqa: E402
from concourse import bacc, mybir  # noqa: E402
from concourse import bass_utils  # noqa: E402

N = 2048          # nodes
NC = 8            # cores
SH = N // NC      # 256 rows per core
IT = SH // 128    # 2 i-tiles per core
H = 3             # heads
E = 3             # raw edge features
F32 = mybir.dt.float32
BF16 = mybir.dt.bfloat16
I32 = mybir.dt.int32
AF = mybir.ActivationFunctionType
OP = mybir.AluOpType

_CACHE = {}


def _host_consts(params):
    """Derive per-layer constant matrices from the GAT params (host math)."""
    P = {k: ({kk: np.asarray(vv, np.float32) for kk, vv in v.items()}
             if isinstance(v, dict) else np.float32(np.asarray(v)))
         for k, v in params.items()}
    L = []
    for li in range(5):
        p = P[f"l{li}"]
        We, be, a = p["We"], p["be"], p["a"]
        c = We.shape[0] // H                      # 3 for l0..3, 1 for l4
        if li == 0:
            a1 = a2 = None
            a3 = a                                 # [H, c]
        else:
            a1, a2, a3 = a[:, :c], a[:, c:2 * c], a[:, 2 * c:]
        W3 = np.zeros((E, H), np.float32)
        b3 = np.zeros((H,), np.float32)
        for h in range(H):
            for cc in range(c):
                W3[:, h] += a3[h, cc] * We[h * c + cc, :]
                b3[h] += a3[h, cc] * be[h * c + cc]
        WeBD = np.zeros((H * E, H * c), np.float32)
        for h in range(H):
            for cc in range(c):
                WeBD[h * E:(h + 1) * E, h * c + cc] = We[h * c + cc, :]
        ent = dict(c=c, W3=W3, b3=b3, WeBD=WeBD, be=be)
        if li > 0:
            Wn, bn = p["Wn"], p["bn"]             # [H*c, 9]
            d_in = Wn.shape[1]
            A1 = np.zeros((H, d_in), np.float32)
            A2 = np.zeros((H, d_in), np.float32)
            c1 = np.zeros((H,), np.float32)
            c2 = np.zeros((H,), np.float32)
            for h in range(H):
                for cc in range(c):
                    A1[h] += a1[h, cc] * Wn[h * c + cc]
                    A2[h] += a2[h, cc] * Wn[h * c + cc]
                    c1[h] += a1[h, cc] * bn[h * c + cc]
                    c2[h] += a2[h, cc] * bn[h * c + cc]
            c1 = c1 + b3                           # fold b3 into the a1 shift
            HCn = H * c
            PROJ = np.zeros((d_in, HCn + 6), np.float32)
            PROJ[:, :HCn] = Wn.T
            PROJ[:, HCn:HCn + 3] = A2.T
            PROJ[:, HCn + 3:] = A1.T
            BIAS = np.concatenate([bn, c2, c1]).astype(np.float32)
            ent.update(PROJ=PROJ, BIAS=BIAS, HCn=HCn)
        L.append(ent)
    pw = [float(np.asarray(P[f"p{i}"])) for i in range(4)]
    return L, pw


def _pack_wconst(L):
    blob, off = [], {}

    def put(name, arr):
        a = np.asarray(arr, np.float32).ravel()
        off[name] = (sum(x.size for x in blob), a.size)
        blob.append(a)

    for li in range(5):
        put(f"WeBD{li}", L[li]["WeBD"])
        c = L[li]["c"]
        for h in range(H):
            put(f"be{li}_{h}", L[li]["be"][h * c:(h + 1) * c])
        if li > 0:
            put(f"PROJ{li}", L[li]["PROJ"])
            put(f"BIAS{li}", L[li]["BIAS"])
    put("third", np.full((H, 1), 1.0 / H, np.float32))
    return np.concatenate(blob), off


def _build(L, pw, woff, wconst_len):
    nc = bacc.Bacc("TRN2", target_bir_lowering=False, debug=False,
                   num_devices=NC)
    ef_d = nc.dram_tensor("ef", [SH, N * E], F32, kind="ExternalInput").ap()
    adj_d = nc.dram_tensor("adj", [SH, N], I32, kind="ExternalInput").ap()
    wc_d = nc.dram_tensor("wc", [1, wconst_len], F32, kind="ExternalInput").ap()
    id_d = nc.dram_tensor("ident", [128, 128], F32, kind="ExternalInput").ap()
    out_d = nc.dram_tensor("out", [1, SH], F32, kind="ExternalOutput").ap()

    def wdram(name, shape):
        o, sz = woff[name]
        assert sz == int(np.prod(shape)), (name, shape, sz)
        ap = wc_d[0, o:o + sz]
        if len(shape) == 2:
            ap = ap.rearrange("(a b) -> a b", b=shape[1])
        else:
            ap = ap.rearrange("(a b) -> a b", b=1)
        return ap

    with tile.TileContext(nc) as tc:
        _emit(tc, nc, ef_d, adj_d, wdram, id_d, out_d, L, pw)
    nc.compile()
    return nc


def _emit(tc, nc, ef_d, adj_d, wdram, id_d, out_d, L, pw):
    from contextlib import ExitStack
    ctx = ExitStack()
    with ctx:
        persist = ctx.enter_context(tc.tile_pool(name="persist", bufs=1))
        consts = ctx.enter_context(tc.tile_pool(name="consts", bufs=1))
        a2pool = ctx.enter_context(tc.tile_pool(name="a2pool", bufs=1))
        lpool = ctx.enter_context(tc.tile_pool(name="lpool", bufs=3))
        upool = ctx.enter_context(tc.tile_pool(name="upool", bufs=3))
        ppool = ctx.enter_context(tc.tile_pool(name="ppool", bufs=2))
        mpool = ctx.enter_context(tc.tile_pool(name="mpool", bufs=2))
        tpool = ctx.enter_context(tc.tile_pool(name="tpool", bufs=2))
        ptpool = ctx.enter_context(tc.tile_pool(name="ptpool", bufs=3))
        smpool = ctx.enter_context(tc.tile_pool(name="smpool", bufs=1))
        accpool = ctx.enter_context(tc.tile_pool(name="accpool", bufs=2))
        psum = ctx.enter_context(tc.tile_pool(name="psum", bufs=1, space="PSUM"))
        psum_t = ctx.enter_context(tc.tile_pool(name="psum_t", bufs=1,
                                                space="PSUM"))
        dram = ctx.enter_context(tc.tile_pool(name="dram", bufs=1, space="DRAM"))

        ident = consts.tile([128, 128], F32)
        nc.sync.dma_start(ident[:], id_d[:])
        third_sb = consts.tile([H, 1], F32, tag="third")
        nc.sync.dma_start(third_sb[:], wdram("third", (H, 1)))

        # per-layer small consts into SBUF
        webd_sb, be_sb, proj_sb, bias_sb = {}, {}, {}, {}
        for li in range(5):
            hc = H * L[li]["c"]
            t = consts.tile([9, 16], F32, tag=f"webd{li}", name=f"webd{li}")
            nc.sync.dma_start(t[:, :hc], wdram(f"WeBD{li}", (9, hc)))
            webd_sb[li] = t
            cch = L[li]["c"]
            for h in range(H):
                t = consts.tile([3, 1], F32, tag=f"be{li}_{h}",
                                name=f"be{li}_{h}")
                nc.sync.dma_start(t[:cch, :], wdram(f"be{li}_{h}", (cch, 1)))
                be_sb[(li, h)] = t
            if li > 0:
                ncol = L[li]["HCn"] + 6
                t = consts.tile([9, 16], F32, tag=f"proj{li}", name=f"proj{li}")
                nc.sync.dma_start(t[:, :ncol], wdram(f"PROJ{li}", (9, ncol)))
                proj_sb[li] = t
                t = consts.tile([16, 1], F32, tag=f"bias{li}", name=f"bias{li}")
                nc.sync.dma_start(t[:ncol, :], wdram(f"BIAS{li}", (ncol, 1)))
                bias_sb[li] = t

        # ---- one-time prep: ef planes (bf16) + adjacency mask (bf16) ----
        efpl = persist.tile([128, IT, E, N], BF16)
        adjm = persist.tile([128, IT, N], BF16)
        with tc.tile_pool(name="stage", bufs=1) as stage:
            efv = ef_d.rearrange("(it p) (hf q) -> it p hf q", p=128, hf=2)
            adv = adj_d.rearrange("(it p) q -> it p q", p=128)
            for it in range(IT):
                for hf in range(2):
                    efst = stage.tile([128, N * E // 2], F32, tag="efst")
                    nc.sync.dma_start(efst[:], efv[it, :, hf, :])
                    src = efst[:].rearrange("p (j e) -> p j e", e=E)
                    for e in range(E):
                        nc.scalar.copy(
                            efpl[:, it, e, hf * (N // 2):(hf + 1) * (N // 2)],
                            src[:, :, e])
                adst = stage.tile([128, N], I32, tag="adst")
                nc.sync.dma_start(adst[:], adv[it])
                nc.vector.tensor_copy(adjm[:, it, :], adst[:])

        # persistent cross-layer state
        x_new = persist.tile([9, SH], F32)           # [hc, i_local]
        alpha1 = persist.tile([128, IT, H], F32)     # local a1 shifts (+b3)
        alpha1n = persist.tile([128, IT, H], F32)    # negated shifts
        ntr = persist.tile([128, NC, 2, 9], BF16)    # n by j-partition
        pay = dram.tile([1, 3072], BF16)
        gath = dram.tile([1, NC * 3072], BF16)

        for li in range(5):
            ly = L[li]
            c = ly["c"]
            W3, b3 = ly["W3"], ly["b3"]
            has_node = li > 0
            hc = H * c

            # ---- consume gathered node data (layers 1..4) ----
            if has_node:
                gv = gath[0, :].rearrange("(m q) -> m q", q=3072)
                for b in range(2):
                    nc.sync.dma_start(
                        ntr[:, :, b, :hc],
                        gv[:, b * 128 * hc:(b + 1) * 128 * hc].rearrange(
                            "m (p q) -> p m q", p=128))
                a2m = a2pool.tile([128, H, N], BF16)
                for h in range(H):
                    src = gv[:, 2304 + h * SH:2304 + (h + 1) * SH]
                    bsrc = bass.AP(tensor=src.tensor, offset=src.offset,
                                   ap=[[0, 128]] + list(src.ap))
                    nc.sync.dma_start(
                        a2m[:, h, :].rearrange("p (m i) -> p m i", m=NC),
                        bsrc)

            # accumulators: cols 0:9 = S (h-major,E), 9:12 = Z
            sacc = accpool.tile([128, IT, 12], F32)
            tps = ([psum_t.tile([4, SH], F32, tag=f"tps{h}",
                                 name=f"tps{li}_{h}")
                    for h in range(H)] if has_node else None)

            for it in range(IT):
                lrs = []
                for h in range(H):
                    lg = lpool.tile([128, N], BF16)
                    if has_node:
                        nc.vector.scalar_tensor_tensor(
                            out=lg[:], in0=efpl[:, it, 0, :],
                            scalar=float(W3[0, h]), in1=a2m[:, h, :],
                            op0=OP.mult, op1=OP.add)
                    else:
                        nc.vector.tensor_scalar(
                            out=lg[:], in0=efpl[:, it, 0, :],
                            scalar1=float(W3[0, h]), scalar2=float(b3[h]),
                            op0=OP.mult, op1=OP.add)
                    lg2 = lpool.tile([128, N], BF16)
                    nc.vector.scalar_tensor_tensor(
                        out=lg2[:], in0=efpl[:, it, 1, :],
                        scalar=float(W3[1, h]), in1=lg[:],
                        op0=OP.mult, op1=OP.add)
                    lg3 = lpool.tile([128, N], BF16)
                    nc.vector.scalar_tensor_tensor(
                        out=lg3[:], in0=efpl[:, it, 2, :],
                        scalar=float(W3[2, h]), in1=lg2[:],
                        op0=OP.mult, op1=OP.add)
                    lr = upool.tile([128, N], BF16, tag="lr", name="lr")
                    if has_node:
                        r1 = upool.tile([128, N], BF16, tag="r1", name="r1")
                        nc.scalar.activation(
                            r1[:], lg3[:], AF.Relu,
                            bias=alpha1[:, it, h:h + 1], scale=1.0)
                        r2 = upool.tile([128, N], BF16, tag="r2", name="r2")
                        nc.scalar.activation(
                            r2[:], lg3[:], AF.Relu,
                            bias=alpha1n[:, it, h:h + 1], scale=-1.0)
                        # lrelu(y) = relu(y) - 0.2*relu(-y)
                        nc.vector.scalar_tensor_tensor(
                            out=lr[:], in0=r2[:], scalar=-0.2, in1=r1[:],
                            op0=OP.mult, op1=OP.add)
                    else:
                        nc.vector.scalar_tensor_tensor(
                            out=lr[:], in0=lg3[:], scalar=0.2, in1=lg3[:],
                            op0=OP.mult, op1=OP.max)
                    lrs.append(lr)
                for h in range(H):
                    pt = ppool.tile([128, N], BF16)
                    nc.scalar.activation(pt[:], lrs[h][:], AF.Exp)
                    ph = mpool.tile([128, N], BF16)
                    nc.vector.scalar_tensor_tensor(
                        out=ph[:], in0=pt[:], scalar=1.0,
                        in1=adjm[:, it, :], op0=OP.mult, op1=OP.mult,
                        accum_out=sacc[:, it, 9 + h:10 + h])
                    for e in range(E):
                        tr = tpool.tile([128, N], BF16)
                        nc.vector.scalar_tensor_tensor(
                            out=tr[:], in0=ph[:], scalar=1.0,
                            in1=efpl[:, it, e, :], op0=OP.mult, op1=OP.mult,
                            accum_out=sacc[:, it, h * E + e:h * E + e + 1])
                    if has_node:
                        pht = ptpool.tile([128, NC * 2, 128], BF16)
                        nc.sync.dma_start_transpose(pht[:], ph[:])
                        for blk in range(NC * 2):
                            nc.tensor.matmul(
                                tps[h][:c, it * 128:(it + 1) * 128],
                                ntr[:, blk // 2, blk % 2,
                                    h * c:(h + 1) * c],
                                pht[:, blk, :],
                                start=(blk == 0), stop=(blk == NC * 2 - 1))

            # ---- assembly (per-head, partition-base-0 tiles) ----
            zinv = accpool.tile([128, IT, H], F32, tag="zinv")
            nc.vector.reciprocal(zinv[:, :, :], sacc[:, :, 9:12])
            zrep = accpool.tile([128, IT, hc], F32, tag="zrep")
            if c > 1:
                zv = zinv[:, :, :]
                src = bass.AP(tensor=zv.tensor, offset=zv.offset,
                              ap=list(zv.ap) + [[0, c]])
                nc.vector.tensor_copy(
                    zrep[:].rearrange("p a (h cc) -> p a h cc", cc=c), src)
            else:
                nc.vector.tensor_copy(zrep[:], zinv[:, :, :])
            st_sb = smpool.tile([9, SH], F32)
            for it in range(IT):
                st_ps = psum.tile([12, 128], F32, tag="stp")
                nc.tensor.transpose(st_ps[:9, :], sacc[:, it, 0:9], ident[:])
                nc.scalar.copy(st_sb[:, it * 128:(it + 1) * 128], st_ps[:9, :])
            zts = []
            for h in range(H):
                zt_sb = smpool.tile([4, SH], F32, tag=f"zts{h}",
                                    name=f"zts{li}_{h}")
                for it in range(IT):
                    zt_ps = psum.tile([4, 128], F32, tag="ztp")
                    nc.tensor.transpose(zt_ps[:c, :],
                                        zrep[:, it, h * c:(h + 1) * c],
                                        ident[:])
                    nc.scalar.copy(zt_sb[:c, it * 128:(it + 1) * 128],
                                   zt_ps[:c, :])
                zts.append(zt_sb)
            xnh = []
            for h in range(H):
                wes_ps = psum.tile([4, SH], F32, tag="asm")
                nc.tensor.matmul(wes_ps[:c, :],
                                 webd_sb[li][:, h * c:(h + 1) * c],
                                 st_sb[:], start=True, stop=True)
                xp = smpool.tile([4, SH], F32, tag="xp")
                nc.scalar.copy(xp[:c, :], wes_ps[:c, :])
                if has_node:
                    xs = smpool.tile([4, SH], F32, tag="xs")
                    nc.vector.scalar_tensor_tensor(
                        out=xs[:c, :], in0=tps[h][:c, :], scalar=1.0,
                        in1=xp[:c, :], op0=OP.mult, op1=OP.add)
                else:
                    xs = xp
                xz = smpool.tile([4, SH], F32, tag="xz")
                nc.vector.tensor_tensor(xz[:c, :], xs[:c, :], zts[h][:c, :],
                                        op=OP.mult)
                xnh.append(xz)
            if li < 4:
                w = pw[li]
                for h in range(H):
                    beh = be_sb[(li, h)]
                    xz = xnh[h]
                    u2 = smpool.tile([4, SH], F32, tag="u3")
                    nc.vector.tensor_scalar(
                        out=u2[:c, :], in0=xz[:c, :], scalar1=beh[:c, :],
                        scalar2=w, op0=OP.add, op1=OP.mult)
                    xn = smpool.tile([4, SH], F32, tag="xn")
                    nc.vector.scalar_tensor_tensor(
                        out=xn[:c, :], in0=xz[:c, :], scalar=beh[:c, :],
                        in1=u2[:c, :], op0=OP.add, op1=OP.max)
                    nc.sync.dma_start(x_new[h * c:(h + 1) * c, :], xn[:c, :])

                nly = L[li + 1]
                nHC = nly["HCn"]
                ncol = nHC + 6
                r_ps = psum.tile([16, SH], F32, tag="asm")
                nc.tensor.matmul(r_ps[:ncol, :], proj_sb[li + 1][:, :ncol],
                                 x_new[:], start=True, stop=True)
                r_sb = smpool.tile([16, SH], F32, tag="rsb")
                nc.scalar.activation(r_sb[:ncol, :], r_ps[:ncol, :],
                                     AF.Identity,
                                     bias=bias_sb[li + 1][:ncol, :])
                a2b = smpool.tile([16, SH], BF16, tag="a2b")
                nc.vector.tensor_copy(a2b[:ncol, :], r_sb[:ncol, :])
                nc.sync.dma_start(
                    pay[0, 2304:2304 + H * SH].rearrange(
                        "(h i) -> h i", h=H), a2b[nHC:nHC + 3, :])
                for it in range(IT):
                    rt_ps = psum.tile([128, 16], F32, tag="rtp")
                    nc.tensor.transpose(
                        rt_ps[:, :ncol],
                        r_sb[:ncol, it * 128:(it + 1) * 128],
                        ident[:ncol, :ncol])
                    nc.scalar.copy(alpha1[:, it, :],
                                   rt_ps[:, nHC + 3:nHC + 6])
                    nc.vector.tensor_scalar(
                        out=alpha1n[:, it, :], in0=alpha1[:, it, :],
                        scalar1=-1.0, scalar2=None, op0=OP.mult)
                    nrm = smpool.tile([128, 16], BF16, tag="nrm")
                    nc.vector.tensor_copy(nrm[:, :nHC], rt_ps[:, :nHC])
                    nc.sync.dma_start(
                        pay[0, it * 128 * nHC:(it + 1) * 128 * nHC].rearrange(
                            "(p q) -> p q", p=128), nrm[:, :nHC])
                nc.gpsimd.collective_compute(
                    "AllGather", OP.bypass,
                    replica_groups=[list(range(NC))],
                    ins=[pay[:].opt()], outs=[gath[:].opt()])
            else:
                xh3 = smpool.tile([H, SH], F32, tag="xh3")
                for h in range(H):
                    xb = smpool.tile([4, SH], F32, tag="xb")
                    nc.vector.tensor_scalar(
                        out=xb[:c, :], in0=xnh[h][:c, :],
                        scalar1=be_sb[(li, h)][:c, :],
                        scalar2=None, op0=OP.add)
                    nc.sync.dma_start(xh3[h:h + 1, :], xb[:c, :])
                mn_ps = psum.tile([1, SH], F32, tag="mn")
                nc.tensor.matmul(mn_ps[:], third_sb[:], xh3[:],
                                 start=True, stop=True)
                x4 = smpool.tile([1, SH], F32, tag="x4")
                nc.scalar.copy(x4[:], mn_ps[:])
                zz = smpool.tile([1, SH], F32, tag="zz")
                nc.vector.tensor_tensor(zz[:], x4[:], x4[:], op=OP.subtract)
                ex = smpool.tile([1, SH], F32, tag="ex")
                nc.scalar.activation(ex[:], zz[:], AF.Exp)
                exr = smpool.tile([1, SH], F32, tag="exr")
                nc.vector.reciprocal(exr[:], ex[:])
                ones = smpool.tile([1, SH], F32, tag="ones")
                nc.vector.tensor_tensor(ones[:], ex[:], exr[:], op=OP.mult)
                nc.sync.dma_start(out_d[:], ones[:])


def _get_program(params):
    key = "prog"
    if key not in _CACHE:
        L, pw = _host_consts(params)
        wc, woff = _pack_wconst(L)
        nc = _build(L, pw, woff, wc.size)
        _CACHE[key] = (nc, wc)
    return _CACHE[key]


def make_in_maps(edge_feats, adj_matrix, params):
    ef = np.asarray(edge_feats, np.float32).reshape(N, N * E)
    adj = np.ascontiguousarray(np.asarray(adj_matrix, np.int32).reshape(N, N))
    nc, wc = _get_program(params)
    ident = np.eye(128, dtype=np.float32)
    in_maps = []
    for k in range(NC):
        in_maps.append({
            "ef": np.ascontiguousarray(ef[k * SH:(k + 1) * SH]),
            "adj": adj[k * SH:(k + 1) * SH],
            "wc": wc.reshape(1, -1),
            "ident": ident,
        })
    return nc, in_maps


def kernel(edge_feats, adj_matrix, params):
    nc, in_maps = make_in_maps(edge_feats, adj_matrix, params)
    res = bass_utils.run_bass_kernel_spmd(nc, in_maps, core_ids=list(range(NC)))
    out = np.concatenate([res.results[k]["out"][0] for k in range(NC)])
    return out.reshape(1, N, 1).astype(np.float32)


# revision 16
# speedup vs baseline: 22.2758x; 2.1053x over previous
"""Trainium2 Bass kernel for nn_GAT_68642167324863 (5-layer dense GAT, N=2048).

Self-contained: hardcodes shapes/sharding. Shards the NxN edge grid by
destination row across 8 NeuronCores; node features are all-gathered between
layers via a DRAM-bounce AllGather collective.
"""
import sys
import numpy as np

sys.path.insert(0, "/opt/trn_rl_repo")

import concourse.bass as bass  # noqa: E402
import concourse.tile as tile  # noqa: E402
from concourse import bacc, mybir  # noqa: E402
from concourse import bass_utils  # noqa: E402

N = 2048          # nodes
NC = 8            # cores
SH = N // NC      # 256 rows per core
IT = SH // 128    # 2 i-tiles per core
H = 3             # heads
E = 3             # raw edge features
F32 = mybir.dt.float32
BF16 = mybir.dt.bfloat16
I32 = mybir.dt.int32
AF = mybir.ActivationFunctionType
OP = mybir.AluOpType

_CACHE = {}


def _host_consts(params):
    """Derive per-layer constant matrices from the GAT params (host math)."""
    P = {k: ({kk: np.asarray(vv, np.float32) for kk, vv in v.items()}
             if isinstance(v, dict) else np.float32(np.asarray(v)))
         for k, v in params.items()}
    L = []
    for li in range(5):
        p = P[f"l{li}"]
        We, be, a = p["We"], p["be"], p["a"]
        c = We.shape[0] // H                      # 3 for l0..3, 1 for l4
        if li == 0:
            a1 = a2 = None
            a3 = a                                 # [H, c]
        else:
            a1, a2, a3 = a[:, :c], a[:, c:2 * c], a[:, 2 * c:]
        W3 = np.zeros((E, H), np.float32)
        b3 = np.zeros((H,), np.float32)
        for h in range(H):
            for cc in range(c):
                W3[:, h] += a3[h, cc] * We[h * c + cc, :]
                b3[h] += a3[h, cc] * be[h * c + cc]
        WeBD = np.zeros((H * E, H * c), np.float32)
        for h in range(H):
            for cc in range(c):
                WeBD[h * E:(h + 1) * E, h * c + cc] = We[h * c + cc, :]
        ent = dict(c=c, W3=W3, b3=b3, WeBD=WeBD, be=be)
        if li > 0:
            Wn, bn = p["Wn"], p["bn"]             # [H*c, 9]
            d_in = Wn.shape[1]
            A1 = np.zeros((H, d_in), np.float32)
            A2 = np.zeros((H, d_in), np.float32)
            c1 = np.zeros((H,), np.float32)
            c2 = np.zeros((H,), np.float32)
            for h in range(H):
                for cc in range(c):
                    A1[h] += a1[h, cc] * Wn[h * c + cc]
                    A2[h] += a2[h, cc] * Wn[h * c + cc]
                    c1[h] += a1[h, cc] * bn[h * c + cc]
                    c2[h] += a2[h, cc] * bn[h * c + cc]
            c1 = c1 + b3                           # fold b3 into the a1 shift
            HCn = H * c
            PROJ = np.zeros((d_in, HCn + 6), np.float32)
            PROJ[:, :HCn] = Wn.T
            PROJ[:, HCn:HCn + 3] = A2.T
            PROJ[:, HCn + 3:] = A1.T
            BIAS = np.concatenate([bn, c2, c1]).astype(np.float32)
            ent.update(PROJ=PROJ, BIAS=BIAS, HCn=HCn)
        L.append(ent)
    pw = [float(np.asarray(P[f"p{i}"])) for i in range(4)]
    return L, pw


def _pack_wconst(L):
    blob, off = [], {}

    def put(name, arr):
        a = np.asarray(arr, np.float32).ravel()
        off[name] = (sum(x.size for x in blob), a.size)
        blob.append(a)

    for li in range(5):
        put(f"WeBD{li}", L[li]["WeBD"])
        c = L[li]["c"]
        for h in range(H):
            put(f"be{li}_{h}", L[li]["be"][h * c:(h + 1) * c])
        if li > 0:
            put(f"PROJ{li}", L[li]["PROJ"])
            put(f"BIAS{li}", L[li]["BIAS"])
    put("third", np.full((H, 1), 1.0 / H, np.float32))
    return np.concatenate(blob), off


def _build(L, pw, woff, wconst_len):
    nc = bacc.Bacc("TRN2", target_bir_lowering=False, debug=False,
                   num_devices=NC)
    ef_d = nc.dram_tensor("ef", [SH, N * E], F32, kind="ExternalInput").ap()
    adj_d = nc.dram_tensor("adj", [SH, N], I32, kind="ExternalInput").ap()
    wc_d = nc.dram_tensor("wc", [1, wconst_len], F32, kind="ExternalInput").ap()
    id_d = nc.dram_tensor("ident", [128, 128], F32, kind="ExternalInput").ap()
    out_d = nc.dram_tensor("out", [1, SH], F32, kind="ExternalOutput").ap()

    def wdram(name, shape):
        o, sz = woff[name]
        assert sz == int(np.prod(shape)), (name, shape, sz)
        ap = wc_d[0, o:o + sz]
        if len(shape) == 2:
            ap = ap.rearrange("(a b) -> a b", b=shape[1])
        else:
            ap = ap.rearrange("(a b) -> a b", b=1)
        return ap

    with tile.TileContext(nc) as tc:
        _emit(tc, nc, ef_d, adj_d, wdram, id_d, out_d, L, pw)
    nc.compile()
    return nc


def _emit(tc, nc, ef_d, adj_d, wdram, id_d, out_d, L, pw):
    from contextlib import ExitStack
    ctx = ExitStack()
    with ctx:
        persist = ctx.enter_context(tc.tile_pool(name="persist", bufs=1))
        consts = ctx.enter_context(tc.tile_pool(name="consts", bufs=1))
        a2pool = ctx.enter_context(tc.tile_pool(name="a2pool", bufs=1))
        lpool = ctx.enter_context(tc.tile_pool(name="lpool", bufs=3))
        upool = ctx.enter_context(tc.tile_pool(name="upool", bufs=3))
        ppool = ctx.enter_context(tc.tile_pool(name="ppool", bufs=2))
        mpool = ctx.enter_context(tc.tile_pool(name="mpool", bufs=2))
        tpool = ctx.enter_context(tc.tile_pool(name="tpool", bufs=2))
        ptpool = ctx.enter_context(tc.tile_pool(name="ptpool", bufs=2))
        smpool = ctx.enter_context(tc.tile_pool(name="smpool", bufs=2))
        accpool = ctx.enter_context(tc.tile_pool(name="accpool", bufs=2))
        psum = ctx.enter_context(tc.tile_pool(name="psum", bufs=1, space="PSUM"))
        psum_t = ctx.enter_context(tc.tile_pool(name="psum_t", bufs=1,
                                                space="PSUM"))
        dram = ctx.enter_context(tc.tile_pool(name="dram", bufs=1, space="DRAM"))

        ident = consts.tile([128, 128], F32)
        nc.sync.dma_start(ident[:], id_d[:])
        third_sb = consts.tile([H, 1], F32, tag="third")
        nc.sync.dma_start(third_sb[:], wdram("third", (H, 1)))

        # per-layer small consts into SBUF
        webd_sb, be_sb, proj_sb, bias_sb = {}, {}, {}, {}
        for li in range(5):
            hc = H * L[li]["c"]
            t = consts.tile([9, 16], F32, tag=f"webd{li}", name=f"webd{li}")
            nc.sync.dma_start(t[:, :hc], wdram(f"WeBD{li}", (9, hc)))
            webd_sb[li] = t
            cch = L[li]["c"]
            for h in range(H):
                t = consts.tile([3, 1], F32, tag=f"be{li}_{h}",
                                name=f"be{li}_{h}")
                nc.sync.dma_start(t[:cch, :], wdram(f"be{li}_{h}", (cch, 1)))
                be_sb[(li, h)] = t
            if li > 0:
                ncol = L[li]["HCn"] + 6
                t = consts.tile([9, 16], F32, tag=f"proj{li}", name=f"proj{li}")
                nc.sync.dma_start(t[:, :ncol], wdram(f"PROJ{li}", (9, ncol)))
                proj_sb[li] = t
                t = consts.tile([16, 1], F32, tag=f"bias{li}", name=f"bias{li}")
                nc.sync.dma_start(t[:ncol, :], wdram(f"BIAS{li}", (ncol, 1)))
                bias_sb[li] = t

        # ---- one-time prep: ef planes (bf16) + adjacency mask (bf16) ----
        efpl = persist.tile([128, IT, E, N], BF16)
        adjm = persist.tile([128, IT, N], BF16)
        with tc.tile_pool(name="stage", bufs=1) as stage:
            efv = ef_d.rearrange("(it p) (hf q) -> it p hf q", p=128, hf=2)
            adv = adj_d.rearrange("(it p) q -> it p q", p=128)
            for it in range(IT):
                for hf in range(2):
                    efst = stage.tile([128, N * E // 2], F32, tag="efst")
                    nc.sync.dma_start(efst[:], efv[it, :, hf, :])
                    src = efst[:].rearrange("p (j e) -> p j e", e=E)
                    for e in range(E):
                        nc.scalar.copy(
                            efpl[:, it, e, hf * (N // 2):(hf + 1) * (N // 2)],
                            src[:, :, e])
                adst = stage.tile([128, N], I32, tag="adst")
                nc.sync.dma_start(adst[:], adv[it])
                nc.vector.tensor_copy(adjm[:, it, :], adst[:])

        # persistent cross-layer state
        x_new = persist.tile([9, SH], F32)           # [hc, i_local]
        alpha1 = persist.tile([128, IT, H], F32)     # local a1 shifts (+b3)
        alpha1n = persist.tile([128, IT, H], F32)    # negated shifts
        ntr = persist.tile([128, NC, 2, 9], BF16)    # n by j-partition
        pay = dram.tile([1, 3072], BF16)
        gath = dram.tile([1, NC * 3072], BF16)

        for li in range(5):
            ly = L[li]
            c = ly["c"]
            W3, b3 = ly["W3"], ly["b3"]
            has_node = li > 0
            hc = H * c

            # ---- consume gathered node data (layers 1..4) ----
            if has_node:
                gv = gath[0, :].rearrange("(m q) -> m q", q=3072)
                for b in range(2):
                    nc.sync.dma_start(
                        ntr[:, :, b, :hc],
                        gv[:, b * 128 * hc:(b + 1) * 128 * hc].rearrange(
                            "m (p q) -> p m q", p=128))
                a2m = a2pool.tile([128, H, N], BF16)
                for h in range(H):
                    src = gv[:, 2304 + h * SH:2304 + (h + 1) * SH]
                    bsrc = bass.AP(tensor=src.tensor, offset=src.offset,
                                   ap=[[0, 128]] + list(src.ap))
                    nc.sync.dma_start(
                        a2m[:, h, :].rearrange("p (m i) -> p m i", m=NC),
                        bsrc)

            # accumulators: cols 0:9 = S (h-major,E), 9:12 = Z
            sacc = accpool.tile([128, IT, 12], F32)
            tps = ([psum_t.tile([4, SH], F32, tag=f"tps{h}",
                                 name=f"tps{li}_{h}")
                    for h in range(H)] if has_node else None)

            for it in range(IT):
                lrs = []
                for h in range(H):
                    lg = lpool.tile([128, N], BF16)
                    if has_node:
                        nc.vector.scalar_tensor_tensor(
                            out=lg[:], in0=efpl[:, it, 0, :],
                            scalar=float(W3[0, h]), in1=a2m[:, h, :],
                            op0=OP.mult, op1=OP.add)
                    else:
                        nc.vector.tensor_scalar(
                            out=lg[:], in0=efpl[:, it, 0, :],
                            scalar1=float(W3[0, h]), scalar2=float(b3[h]),
                            op0=OP.mult, op1=OP.add)
                    lg2 = lpool.tile([128, N], BF16)
                    nc.vector.scalar_tensor_tensor(
                        out=lg2[:], in0=efpl[:, it, 1, :],
                        scalar=float(W3[1, h]), in1=lg[:],
                        op0=OP.mult, op1=OP.add)
                    lg3 = lpool.tile([128, N], BF16)
                    nc.vector.scalar_tensor_tensor(
                        out=lg3[:], in0=efpl[:, it, 2, :],
                        scalar=float(W3[2, h]), in1=lg2[:],
                        op0=OP.mult, op1=OP.add)
                    lr = upool.tile([128, N], BF16, tag="lr", name="lr")
                    if has_node:
                        r1 = upool.tile([128, N], BF16, tag="r1", name="r1")
                        nc.scalar.activation(
                            r1[:], lg3[:], AF.Relu,
                            bias=alpha1[:, it, h:h + 1], scale=1.0)
                        r2 = upool.tile([128, N], BF16, tag="r2", name="r2")
                        nc.scalar.activation(
                            r2[:], lg3[:], AF.Relu,
                            bias=alpha1n[:, it, h:h + 1], scale=-1.0)
                        # lrelu(y) = relu(y) - 0.2*relu(-y)
                        nc.vector.scalar_tensor_tensor(
                            out=lr[:], in0=r2[:], scalar=-0.2, in1=r1[:],
                            op0=OP.mult, op1=OP.add)
                    else:
                        nc.vector.scalar_tensor_tensor(
                            out=lr[:], in0=lg3[:], scalar=0.2, in1=lg3[:],
                            op0=OP.mult, op1=OP.max)
                    lrs.append(lr)
                for h in range(H):
                    pt = ppool.tile([128, N], BF16)
                    nc.scalar.activation(pt[:], lrs[h][:], AF.Exp)
                    ph = mpool.tile([128, N], BF16)
                    nc.vector.scalar_tensor_tensor(
                        out=ph[:], in0=pt[:], scalar=1.0,
                        in1=adjm[:, it, :], op0=OP.mult, op1=OP.mult,
                        accum_out=sacc[:, it, 9 + h:10 + h])
                    for e in range(E):
                        tr = tpool.tile([128, N], BF16)
                        nc.vector.scalar_tensor_tensor(
                            out=tr[:], in0=ph[:], scalar=1.0,
                            in1=efpl[:, it, e, :], op0=OP.mult, op1=OP.mult,
                            accum_out=sacc[:, it, h * E + e:h * E + e + 1])
                    if has_node:
                        pht = ptpool.tile([128, NC * 2, 128], BF16)
                        nc.sync.dma_start_transpose(pht[:], ph[:])
                        for blk in range(NC * 2):
                            nc.tensor.matmul(
                                tps[h][:c, it * 128:(it + 1) * 128],
                                ntr[:, blk // 2, blk % 2,
                                    h * c:(h + 1) * c],
                                pht[:, blk, :],
                                start=(blk == 0), stop=(blk == NC * 2 - 1))

            # ---- assembly (per-head, partition-base-0 tiles) ----
            zinv = accpool.tile([128, IT, H], F32, tag="zinv")
            nc.vector.reciprocal(zinv[:, :, :], sacc[:, :, 9:12])
            zrep = accpool.tile([128, IT, hc], F32, tag="zrep")
            if c > 1:
                zv = zinv[:, :, :]
                src = bass.AP(tensor=zv.tensor, offset=zv.offset,
                              ap=list(zv.ap) + [[0, c]])
                nc.vector.tensor_copy(
                    zrep[:].rearrange("p a (h cc) -> p a h cc", cc=c), src)
            else:
                nc.vector.tensor_copy(zrep[:], zinv[:, :, :])
            st_sb = smpool.tile([9, SH], F32)
            for it in range(IT):
                st_ps = psum.tile([12, 128], F32, tag="stp")
                nc.tensor.transpose(st_ps[:9, :], sacc[:, it, 0:9], ident[:])
                nc.scalar.copy(st_sb[:, it * 128:(it + 1) * 128], st_ps[:9, :])
            zts = []
            for h in range(H):
                zt_sb = smpool.tile([4, SH], F32, tag=f"zts{h}",
                                    name=f"zts{li}_{h}")
                for it in range(IT):
                    zt_ps = psum.tile([4, 128], F32, tag="ztp")
                    nc.tensor.transpose(zt_ps[:c, :],
                                        zrep[:, it, h * c:(h + 1) * c],
                                        ident[:])
                    nc.scalar.copy(zt_sb[:c, it * 128:(it + 1) * 128],
                                   zt_ps[:c, :])
                zts.append(zt_sb)
            xnh = []
            for h in range(H):
                wes_ps = psum.tile([4, SH], F32, tag="asm")
                nc.tensor.matmul(wes_ps[:c, :],
                                 webd_sb[li][:, h * c:(h + 1) * c],
                                 st_sb[:], start=True, stop=True)
                xp = smpool.tile([4, SH], F32, tag="xp")
                nc.scalar.copy(xp[:c, :], wes_ps[:c, :])
                if has_node:
                    xs = smpool.tile([4, SH], F32, tag="xs")
                    nc.vector.scalar_tensor_tensor(
                        out=xs[:c, :], in0=tps[h][:c, :], scalar=1.0,
                        in1=xp[:c, :], op0=OP.mult, op1=OP.add)
                else:
                    xs = xp
                xz = smpool.tile([4, SH], F32, tag="xz")
                nc.vector.tensor_tensor(xz[:c, :], xs[:c, :], zts[h][:c, :],
                                        op=OP.mult)
                xnh.append(xz)
            if li < 4:
                w = pw[li]
                for h in range(H):
                    beh = be_sb[(li, h)]
                    xz = xnh[h]
                    u2 = smpool.tile([4, SH], F32, tag="u3")
                    nc.vector.tensor_scalar(
                        out=u2[:c, :], in0=xz[:c, :], scalar1=beh[:c, :],
                        scalar2=w, op0=OP.add, op1=OP.mult)
                    xn = smpool.tile([4, SH], F32, tag="xn")
                    nc.vector.scalar_tensor_tensor(
                        out=xn[:c, :], in0=xz[:c, :], scalar=beh[:c, :],
                        in1=u2[:c, :], op0=OP.add, op1=OP.max)
                    nc.sync.dma_start(x_new[h * c:(h + 1) * c, :], xn[:c, :])

                nly = L[li + 1]
                nHC = nly["HCn"]
                ncol = nHC + 6
                r_ps = psum.tile([16, SH], F32, tag="asm")
                nc.tensor.matmul(r_ps[:ncol, :], proj_sb[li + 1][:, :ncol],
                                 x_new[:], start=True, stop=True)
                r_sb = smpool.tile([16, SH], F32, tag="rsb")
                nc.scalar.activation(r_sb[:ncol, :], r_ps[:ncol, :],
                                     AF.Identity,
                                     bias=bias_sb[li + 1][:ncol, :])
                a2b = smpool.tile([16, SH], BF16, tag="a2b")
                nc.vector.tensor_copy(a2b[:ncol, :], r_sb[:ncol, :])
                nc.sync.dma_start(
                    pay[0, 2304:2304 + H * SH].rearrange(
                        "(h i) -> h i", h=H), a2b[nHC:nHC + 3, :])
                for it in range(IT):
                    rt_ps = psum.tile([128, 16], F32, tag="rtp")
                    nc.tensor.transpose(
                        rt_ps[:, :ncol],
                        r_sb[:ncol, it * 128:(it + 1) * 128],
                        ident[:ncol, :ncol])
                    nc.scalar.copy(alpha1[:, it, :],
                                   rt_ps[:, nHC + 3:nHC + 6])
                    nc.vector.tensor_scalar(
                        out=alpha1n[:, it, :], in0=alpha1[:, it, :],
                        scalar1=-1.0, scalar2=None, op0=OP.mult)
                    nrm = smpool.tile([128, 16], BF16, tag="nrm")
                    nc.vector.tensor_copy(nrm[:, :nHC], rt_ps[:, :nHC])
                    nc.sync.dma_start(
                        pay[0, it * 128 * nHC:(it + 1) * 128 * nHC].rearrange(
                            "(p q) -> p q", p=128), nrm[:, :nHC])
                nc.gpsimd.collective_compute(
                    "AllGather", OP.bypass,
                    replica_groups=[list(range(NC))],
                    ins=[pay[:].opt()], outs=[gath[:].opt()])
            else:
                xh3 = smpool.tile([H, SH], F32, tag="xh3")
                for h in range(H):
                    xb = smpool.tile([4, SH], F32, tag="xb")
                    nc.vector.tensor_scalar(
                        out=xb[:c, :], in0=xnh[h][:c, :],
                        scalar1=be_sb[(li, h)][:c, :],
                        scalar2=None, op0=OP.add)
                    nc.sync.dma_start(xh3[h:h + 1, :], xb[:c, :])
                mn_ps = psum.tile([1, SH], F32, tag="mn")
                nc.tensor.matmul(mn_ps[:], third_sb[:], xh3[:],
                                 start=True, stop=True)
                x4 = smpool.tile([1, SH], F32, tag="x4")
                nc.scalar.copy(x4[:], mn_ps[:])
                zz = smpool.tile([1, SH], F32, tag="zz")
                nc.vector.tensor_tensor(zz[:], x4[:], x4[:], op=OP.subtract)
                ex = smpool.tile([1, SH], F32, tag="ex")
                nc.scalar.activation(ex[:], zz[:], AF.Exp)
                exr = smpool.tile([1, SH], F32, tag="exr")
                nc.vector.reciprocal(exr[:], ex[:])
                ones = smpool.tile([1, SH], F32, tag="ones")
                nc.vector.tensor_tensor(ones[:], ex[:], exr[:], op=OP.mult)
                nc.sync.dma_start(out_d[:], ones[:])


def _get_program(params):
    key = "prog"
    if key not in _CACHE:
        L, pw = _host_consts(params)
        wc, woff = _pack_wconst(L)
        nc = _build(L, pw, woff, wc.size)
        _CACHE[key] = (nc, wc)
    return _CACHE[key]


def make_in_maps(edge_feats, adj_matrix, params):
    ef = np.asarray(edge_feats, np.float32).reshape(N, N * E)
    adj = np.ascontiguousarray(np.asarray(adj_matrix, np.int32).reshape(N, N))
    nc, wc = _get_program(params)
    ident = np.eye(128, dtype=np.float32)
    in_maps = []
    for k in range(NC):
        in_maps.append({
            "ef": np.ascontiguousarray(ef[k * SH:(k + 1) * SH]),
            "adj": adj[k * SH:(k + 1) * SH],
            "wc": wc.reshape(1, -1),
            "ident": ident,
        })
    return nc, in_maps


def kernel(edge_feats, adj_matrix, params):
    nc, in_maps = make_in_maps(edge_feats, adj_matrix, params)
    res = bass_utils.run_bass_kernel_spmd(nc, in_maps, core_ids=list(range(NC)))
    out = np.concatenate([res.results[k]["out"][0] for k in range(NC)])
    return out.reshape(1, N, 1).astype(np.float32)
